# revision 11
# baseline (speedup 1.0000x reference)
"""Trainium2 Bass kernel for nn_Bottleneck_67525475828280.

ResNet bottleneck (conv1x1 -> BN/ReLU -> conv3x3 -> BN/ReLU -> conv1x1 -> BN
-> +residual/ReLU) with per-conv cosine-activation regularization scalar.

Strategy:
  * Data-parallel over batch: 16 images -> 2 per NeuronCore (8 cores).
  * Device computes: the three convs (PE matmuls; conv2 as 6 matmuls/tile via
    a shifted-copy tap-pairing on the zero-padded activation), training-mode
    BN statistics (free-dim accumulation via accum_out; cross-core AllReduce
    of [sum, sum-of-squares]), BN apply + ReLU, residual add.
  * Host computes (gather/unshard glue): BN coefficients are reproduced from
    the returned conv outputs to rebuild z1/z2, giving the "norm" fields for
    the cosine regularization; the jax.random.categorical sampling
    (data-independent PRNG, identical keys to the reference); the box-filter
    means at the five sampled points per (b, channel); the scalar `acc`.

Self-contained: hardcodes shapes/sharding for this problem.
"""

import os
import numpy as np

import concourse.bass as bass
import concourse.mybir as mybir
import concourse.tile as tile
from concourse import bacc
from concourse.bass_utils import run_bass_kernel_spmd

F32 = mybir.dt.float32
F32R = mybir.dt.float32r
AF = mybir.ActivationFunctionType
ALU = mybir.AluOpType

NCORES = 8
B = 16
BLOC = B // NCORES          # 2 images per core
H = W = 56
HW = H * W                  # 3136
NT = 7                      # pixel tiles per image
TN = HW // NT               # 448 = 8 rows x 56
ROWS = TN // W              # 8
PW = W + 2                  # 58 (zero-padded width)
PHW = PW * PW               # 3364
C1, C2, C3 = 256, 64, 256   # conv in/mid/out channels
N_TOT = float(B * HW)       # BN population size
BN_EPS = 1e-5
EPS = 1e-6
TEMP = 0.5
NSAMP = 5
RADIUS = 5

# conv2/conv3 matmul input precision: "f32r" (4x faster PE) or "f32" (exact)
MODE = os.environ.get("BOTTLENECK_MODE", "f32r")

_BUILD_CACHE = {}


def build_nc(mode=MODE):
    if mode in _BUILD_CACHE:
        return _BUILD_CACHE[mode]

    mm_dt = F32R if mode == "f32r" else F32
    cast = mode == "f32r"

    nc = bacc.Bacc("TRN2", target_bir_lowering=False, debug=False,
                   num_devices=NCORES)

    # ---- DRAM I/O (per-core shapes) ----
    xin = nc.dram_tensor("xin", [BLOC, C1, HW], F32, kind="ExternalInput").ap()
    w1e = nc.dram_tensor("w1e", [2, 128, 64], F32, kind="ExternalInput").ap()
    w2p = nc.dram_tensor("w2p", [3, 128, 64], F32, kind="ExternalInput").ap()
    w2s = nc.dram_tensor("w2s", [3, 64, 64], F32, kind="ExternalInput").ap()
    w3e = nc.dram_tensor("w3e", [2, 64, 128], F32, kind="ExternalInput").ap()
    g1b1 = nc.dram_tensor("g1b1", [64, 2], F32, kind="ExternalInput").ap()
    g2b2 = nc.dram_tensor("g2b2", [64, 2], F32, kind="ExternalInput").ap()
    g3b3 = nc.dram_tensor("g3b3", [128, 4], F32, kind="ExternalInput").ap()

    y1o = nc.dram_tensor("y1o", [BLOC, 64, HW], F32, kind="ExternalOutput").ap()
    y2o = nc.dram_tensor("y2o", [BLOC, 64, HW], F32, kind="ExternalOutput").ap()
    y3o = nc.dram_tensor("y3o", [BLOC, C3, HW], F32, kind="ExternalOutput").ap()
    outo = nc.dram_tensor("outo", [BLOC, C3, HW], F32, kind="ExternalOutput").ap()

    ts = bass.ts

    with tile.TileContext(nc) as tc:
        with (
            tc.tile_pool(name="const", bufs=1) as cp,
            tc.tile_pool(name="xp", bufs=1) as xp,
            tc.tile_pool(name="work", bufs=1) as wp,
            tc.tile_pool(name="scr", bufs=3) as scr,
            tc.tile_pool(name="dram", bufs=1, space="DRAM") as dp,
        ):
            # ---- conv1 weights (f32) ----
            w1r = [cp.tile([128, 64], F32, name=f"w1r{h}", tag=f"w1r{h}") for h in range(2)]
            for h in range(2):
                nc.sync.dma_start(out=w1r[h][:], in_=w1e[h])

            # ---- x (f32), 7 chunks per (b, half), t-major for early start ----
            xr = [[xp.tile([128, HW], F32, name=f"x{b}{h}", tag=f"x{b}{h}") for h in range(2)]
                  for b in range(BLOC)]
            for b in range(BLOC):
                for t in range(NT):
                    for h in range(2):
                        nc.sync.dma_start(out=xr[b][h][:, ts(t, TN)],
                                          in_=xin[b, 128 * h:128 * (h + 1), ts(t, TN)])

            # ---- conv2/conv3 weights (cast to mm dtype; needed ~90us in) ----
            dma_w = nc.gpsimd.dma_start if cast else nc.sync.dma_start
            w2pr = [cp.tile([128, 64], mm_dt, name=f"w2pr{i}", tag=f"w2pr{i}") for i in range(3)]
            w2sr = [cp.tile([64, 64], mm_dt, name=f"w2sr{i}", tag=f"w2sr{i}") for i in range(3)]
            w3r = [cp.tile([64, 128], mm_dt, name=f"w3r{j}", tag=f"w3r{j}") for j in range(2)]
            for i in range(3):
                dma_w(out=w2pr[i][:], in_=w2p[i])
                dma_w(out=w2sr[i][:], in_=w2s[i])
            for j in range(2):
                dma_w(out=w3r[j][:], in_=w3e[j])

            gb1 = cp.tile([64, 2], F32, name="gb1", tag="gb1")
            gb2 = cp.tile([64, 2], F32, name="gb2", tag="gb2")
            gb3 = cp.tile([128, 4], F32, name="gb3", tag="gb3")
            nc.sync.dma_start(out=gb1[:], in_=g1b1[:])
            nc.sync.dma_start(out=gb2[:], in_=g2b2[:])
            nc.sync.dma_start(out=gb3[:], in_=g3b3[:])

            # persistent work tiles (tag sharing: y1/zpb die before y3 written)
            y1 = [wp.tile([64, HW], F32, name=f"y1_{b}", tag=f"big{b}0") for b in range(BLOC)]
            zpb = [wp.tile([128, PHW], mm_dt, name=f"zpb{b}", tag=f"big{b}1") for b in range(BLOC)]
            y2t = [wp.tile([64, HW], F32, name=f"y2t{b}", tag=f"y2t{b}") for b in range(BLOC)]
            c3r = [wp.tile([64, HW], mm_dt, name=f"c3r{b}", tag=f"c3r{b}") for b in range(BLOC)]
            y3 = [[wp.tile([128, HW], F32, name=f"y3_{b}{h}", tag=f"big{b}{h}") for h in range(2)]
                  for b in range(BLOC)]

            st1 = cp.tile([64, 28], F32, name="st1", tag="st1")
            st2 = cp.tile([64, 28], F32, name="st2", tag="st2")
            st3 = [cp.tile([128, 18], F32, name=f"st3_{h}", tag=f"st3_{h}") for h in range(2)]

            def bn_coeffs(g_ap, b_ap, sy_ap, sq_ap, pdim, tagp):
                """scale/shift from allreduced [sum, sumsq]."""
                v = cp.tile([pdim, 1], F32, name=f"v{tagp}", tag=f"v{tagp}")
                m = cp.tile([pdim, 1], F32, name=f"m{tagp}", tag=f"m{tagp}")
                r = cp.tile([pdim, 1], F32, name=f"r{tagp}", tag=f"r{tagp}")
                t0 = cp.tile([pdim, 1], F32, name=f"t0{tagp}", tag=f"t0{tagp}")
                t1 = cp.tile([pdim, 1], F32, name=f"t1{tagp}", tag=f"t1{tagp}")
                sc = cp.tile([pdim, 1], F32, name=f"sc{tagp}", tag=f"sc{tagp}")
                sh = cp.tile([pdim, 1], F32, name=f"sh{tagp}", tag=f"sh{tagp}")
                inv_n = 1.0 / N_TOT
                nc.vector.tensor_scalar(m[:], sy_ap, inv_n, 0.0, ALU.mult, ALU.add)
                nc.vector.tensor_scalar(v[:], sq_ap, inv_n, 0.0, ALU.mult, ALU.add)
                nc.vector.tensor_tensor(t0[:], m[:], m[:], ALU.mult)
                nc.vector.tensor_tensor(v[:], v[:], t0[:], ALU.subtract)
                nc.vector.tensor_scalar(v[:], v[:], 1.0, BN_EPS, ALU.mult, ALU.add)
                nc.scalar.activation(t0[:], v[:], AF.Sqrt)
                nc.vector.reciprocal(r[:], t0[:])
                # newton step: r = r * (1.5 - 0.5 * v * r * r)
                nc.vector.tensor_tensor(t0[:], r[:], r[:], ALU.mult)
                nc.vector.tensor_tensor(t1[:], v[:], t0[:], ALU.mult)
                nc.vector.tensor_scalar(t1[:], t1[:], -0.5, 1.5, ALU.mult, ALU.add)
                nc.vector.tensor_tensor(r[:], r[:], t1[:], ALU.mult)
                nc.vector.tensor_tensor(sc[:], g_ap, r[:], ALU.mult)
                nc.vector.tensor_tensor(t0[:], m[:], sc[:], ALU.mult)
                nc.vector.tensor_tensor(sh[:], b_ap, t0[:], ALU.subtract)
                return sc, sh

            def allreduce(src_ap, pdim, ncols, tagp):
                bounce_in = dp.tile([pdim, ncols], F32, name=f"ci{tagp}", tag=f"ci{tagp}")
                bounce_out = dp.tile([pdim, ncols], F32, name=f"co{tagp}", tag=f"co{tagp}")
                nc.gpsimd.dma_start(out=bounce_in[:], in_=src_ap)
                nc.gpsimd.collective_compute(
                    "AllReduce", ALU.add,
                    replica_groups=[list(range(NCORES))],
                    ins=[bounce_in.opt()], outs=[bounce_out.opt()],
                )
                dst = cp.tile([pdim, ncols], F32, name=f"ar{tagp}", tag=f"ar{tagp}")
                nc.sync.dma_start(out=dst[:], in_=bounce_out[:])
                return dst

            # ============ conv1 (f32 exact) ============
            with tc.tile_pool(name="ps1", bufs=4, space="PSUM") as ps1p:
                for b in range(BLOC):
                    for t in range(NT):
                        p = ps1p.tile([64, TN], F32, name="p1", tag="p1")
                        nc.tensor.matmul(p[:], w1r[0][:], xr[b][0][:, ts(t, TN)],
                                         start=True, stop=False)
                        nc.tensor.matmul(p[:], w1r[1][:], xr[b][1][:, ts(t, TN)],
                                         start=False, stop=True)
                        nc.vector.tensor_scalar(
                            y1[b][:, ts(t, TN)], p[:], 1.0, 0.0, ALU.mult, ALU.add,
                            accum_out=st1[:, b * NT + t:b * NT + t + 1])
                        sq = scr.tile([64, TN], F32, name="sq1", tag="sq1")
                        nc.scalar.activation(
                            sq[:], p[:], AF.Square,
                            accum_out=st1[:, 14 + b * NT + t:15 + b * NT + t])
                    nc.sync.dma_start(out=y1o[b], in_=y1[b][:])

            sums1 = cp.tile([64, 2], F32, name="sums1", tag="sums1")
            nc.vector.tensor_reduce(sums1[:, 0:1], st1[:, 0:14], mybir.AxisListType.X, ALU.add)
            nc.vector.tensor_reduce(sums1[:, 1:2], st1[:, 14:28], mybir.AxisListType.X, ALU.add)
            ar1 = allreduce(sums1[:], 64, 2, "1")
            sc1, sh1 = bn_coeffs(gb1[:, 0:1], gb1[:, 1:2],
                                 ar1[:, 0:1], ar1[:, 1:2], 64, "1")

            # bn1 apply -> z1 into padded rhs rows 0-63; shifted copy rows 64-127
            for b in range(BLOC):
                nc.gpsimd.memset(zpb[b][:].bitcast(F32) if cast else zpb[b][:], 0.0)
                zp3 = zpb[b][:].rearrange("c (h w) -> c h w", h=PW)
                interior = zp3[0:64, 1:57, 1:57]
                nc.scalar.activation(interior, y1[b][:].rearrange("c (h w) -> c h w", h=H),
                                     AF.Relu, bias=sh1[:], scale=sc1[:])
                # rows 64-127 = rows 0-63 shifted left by one element
                nc.vector.tensor_copy(zpb[b][64:128, 0:PHW - 1], zpb[b][0:64, 1:PHW])

            # ============ conv2: 6 matmuls/tile (3 paired-dw + 3 single) ============
            with tc.tile_pool(name="ps2", bufs=4, space="PSUM") as ps2p:
                for b in range(BLOC):
                    zp3 = zpb[b][:].rearrange("c (h w) -> c h w", h=PW)
                    for t in range(NT):
                        h0 = ROWS * t
                        p = ps2p.tile([64, TN], F32, name="p2", tag="p2")
                        for dh in range(3):
                            rhs = zp3[:, h0 + dh:h0 + dh + ROWS, 0:W]
                            nc.tensor.matmul(p[:], w2pr[dh][:], rhs,
                                             start=(dh == 0), stop=False)
                        for dh in range(3):
                            rhs = zp3[0:64, h0 + dh:h0 + dh + ROWS, 2:2 + W]
                            nc.tensor.matmul(p[:], w2sr[dh][:], rhs,
                                             start=False, stop=(dh == 2))
                        nc.vector.tensor_scalar(
                            y2t[b][:, ts(t, TN)], p[:], 1.0, 0.0, ALU.mult, ALU.add,
                            accum_out=st2[:, b * NT + t:b * NT + t + 1])
                        sq = scr.tile([64, TN], F32, name="sq2", tag="sq2")
                        nc.scalar.activation(
                            sq[:], p[:], AF.Square,
                            accum_out=st2[:, 14 + b * NT + t:15 + b * NT + t])
                    nc.sync.dma_start(out=y2o[b], in_=y2t[b][:])

            sums2 = cp.tile([64, 2], F32, name="sums2", tag="sums2")
            nc.vector.tensor_reduce(sums2[:, 0:1], st2[:, 0:14], mybir.AxisListType.X, ALU.add)
            nc.vector.tensor_reduce(sums2[:, 1:2], st2[:, 14:28], mybir.AxisListType.X, ALU.add)
            ar2 = allreduce(sums2[:], 64, 2, "2")
            sc2, sh2 = bn_coeffs(gb2[:, 0:1], gb2[:, 1:2],
                                 ar2[:, 0:1], ar2[:, 1:2], 64, "2")

            # bn2 apply -> z2
            for b in range(BLOC):
                nc.scalar.activation(c3r[b][:], y2t[b][:],
                                     AF.Relu, bias=sh2[:], scale=sc2[:])

            # ============ conv3 ============
            with tc.tile_pool(name="ps3", bufs=3, space="PSUM") as ps3p:
                for b in range(BLOC):
                    for t in range(NT):
                        pa = ps3p.tile([128, TN], F32, name="p3a", tag="p3a")
                        pb = ps3p.tile([128, TN], F32, name="p3b", tag="p3b")
                        nc.tensor.matmul(pa[:], w3r[0][:], c3r[b][:, ts(t, TN)],
                                         start=True, stop=True)
                        nc.tensor.matmul(pb[:], w3r[1][:], c3r[b][:, ts(t, TN)],
                                         start=True, stop=True)
                        # evict: h0 via DVE (+sum), h1 via ACT (+sum)
                        nc.vector.tensor_scalar(
                            y3[b][0][:, ts(t, TN)], pa[:], 1.0, 0.0, ALU.mult, ALU.add,
                            accum_out=st3[0][:, b * NT + t:b * NT + t + 1])
                        nc.scalar.activation(
                            y3[b][1][:, ts(t, TN)], pb[:], AF.Copy,
                            accum_out=st3[1][:, b * NT + t:b * NT + t + 1])
                    for hh in range(2):
                        nc.sync.dma_start(out=y3o[b, 128 * hh:128 * (hh + 1), :],
                                          in_=y3[b][hh][:])
                    # sum-of-squares: one big pass per (b, half)
                    sqb = scr.tile([128, HW], F32, name="sqb", tag="sqb", bufs=1)
                    nc.vector.tensor_tensor(
                        sqb[:], y3[b][0][:], y3[b][0][:], ALU.mult)
                    nc.vector.tensor_reduce(st3[0][:, 14 + b:15 + b], sqb[:],
                                            mybir.AxisListType.X, ALU.add)
                    sqc = scr.tile([128, HW], F32, name="sqc", tag="sqc", bufs=1)
                    nc.scalar.activation(sqc[:], y3[b][1][:], AF.Square,
                                         accum_out=st3[1][:, 14 + b:15 + b])

            sums3 = cp.tile([128, 4], F32, name="sums3", tag="sums3")
            for hh in range(2):
                nc.vector.tensor_reduce(sums3[:, hh:hh + 1], st3[hh][:, 0:14],
                                        mybir.AxisListType.X, ALU.add)
                nc.vector.tensor_reduce(sums3[:, 2 + hh:3 + hh], st3[hh][:, 14:16],
                                        mybir.AxisListType.X, ALU.add)
            ar3 = allreduce(sums3[:], 128, 4, "3")
            sc3 = [None, None]
            sh3 = [None, None]
            for hh in range(2):
                sc3[hh], sh3[hh] = bn_coeffs(
                    gb3[:, hh:hh + 1], gb3[:, 2 + hh:3 + hh],
                    ar3[:, hh:hh + 1], ar3[:, 2 + hh:3 + hh], 128, f"3{hh}")

            # bn3 apply + residual + relu (in place on y3, chunked), then DMA out
            RC = HW // 2
            for b in range(BLOC):
                for hh in range(2):
                    for cchunk in range(2):
                        sl = slice(cchunk * RC, (cchunk + 1) * RC)
                        yt = y3[b][hh][:, sl]
                        nc.vector.scalar_tensor_tensor(
                            yt, yt, sc3[hh][:], xr[b][hh][:, sl],
                            ALU.mult, ALU.add)
                        nc.scalar.activation(yt, yt, AF.Relu, bias=sh3[hh][:])
                        nc.sync.dma_start(
                            out=outo[b, 128 * hh:128 * (hh + 1), sl], in_=yt)

    nc.compile()
    _BUILD_CACHE[mode] = nc
    return nc


def _prep_weights(w1, w2, w3, g1, b1, g2, b2, g3, b3):
    w1m = w1.reshape(64, 256).T.astype(np.float32)             # (256, 64)
    w1e = np.stack([w1m[:128], w1m[128:]], 0)                  # (2,128,64)
    # conv2: paired taps (dh,0)+(dh,1) over K=128 (zpad ; zpad<<1), single (dh,2)
    w2p = np.zeros((3, 128, 64), np.float32)
    w2s = np.zeros((3, 64, 64), np.float32)
    for dh in range(3):
        w2p[dh, 0:64] = w2[:, :, dh, 0].T
        w2p[dh, 64:128] = w2[:, :, dh, 1].T
        w2s[dh] = w2[:, :, dh, 2].T
    w3m = w3.reshape(256, 64).T.astype(np.float32)             # (64, 256)
    w3e = np.stack([w3m[:, 0:128], w3m[:, 128:256]], 0)        # (2,64,128)
    g1b1 = np.stack([g1, b1], 1).astype(np.float32)
    g2b2 = np.stack([g2, b2], 1).astype(np.float32)
    g3b3 = np.stack([g3[:128], g3[128:], b3[:128], b3[128:]], 1).astype(np.float32)
    return w1e, w2p, w2s, w3e, g1b1, g2b2, g3b3


def _bn_np(y, g, bb):
    """Host replica of device BN (f64 stats; ~1e-9 rel from device values)."""
    m = y.mean(axis=(0, 2), dtype=np.float64)
    v = (y.astype(np.float64) ** 2).mean(axis=(0, 2)) - m * m
    sc = g.astype(np.float64) / np.sqrt(v + BN_EPS)
    sh = bb.astype(np.float64) - m * sc
    return (y * sc[None, :, None].astype(np.float32)
            + sh[None, :, None].astype(np.float32))


def _host_acc(x, w1, w2, w3, g1, b1, g2, b2, y1, y2, y3):
    """Cosine-regularization scalar from device conv outputs."""
    import jax
    import jax.numpy as jnp

    # norm fields
    s1 = (x.reshape(B, C1, HW).astype(np.float64) ** 2).sum(1).astype(np.float32)
    z1 = np.maximum(_bn_np(y1, g1, b1), 0.0).astype(np.float32)     # (B,64,HW)
    q = (z1.astype(np.float64) ** 2).sum(1).reshape(B, H, W)
    qp = np.zeros((B, H + 2, W + 2))
    qp[:, 1:57, 1:57] = q
    s2 = np.zeros((B, H, W))
    for dh in range(3):
        for dw in range(3):
            s2 = s2 + qp[:, dh:dh + H, dw:dw + W]
    s2 = s2.reshape(B, HW).astype(np.float32)
    z2 = np.maximum(_bn_np(y2, g2, b2), 0.0).astype(np.float32)
    s3 = (z2.astype(np.float64) ** 2).sum(1).astype(np.float32)

    def cosine_field(y, w, s):
        wn = np.sqrt((w.reshape(w.shape[0], -1).astype(np.float64) ** 2).sum(1))
        wn = wn.astype(np.float32)
        xn = np.sqrt(s).astype(np.float32)                     # (B, HW)
        return (y / (wn[None, :, None] + EPS) / (xn[:, None, :] + EPS)).astype(np.float32)

    def act_value(cos, key):
        Bc, Cc = cos.shape[0], cos.shape[1]
        logits = cos / TEMP
        with jax.default_device(jax.devices("cpu")[0]):
            idx = np.asarray(jax.random.categorical(
                key, jnp.asarray(logits), axis=-1, shape=(NSAMP, Bc, Cc)))
        c2d = cos.reshape(Bc, Cc, H, W).astype(np.float64)
        sat = np.zeros((Bc, Cc, H + 1, W + 1))
        sat[:, :, 1:, 1:] = c2d.cumsum(2).cumsum(3)
        hh = idx // W
        ww = idx % W
        h1 = np.clip(hh - RADIUS, 0, H)
        h2 = np.clip(hh + RADIUS + 1, 0, H)
        w1_ = np.clip(ww - RADIUS, 0, W)
        w2_ = np.clip(ww + RADIUS + 1, 0, W)
        bi = np.broadcast_to(np.arange(Bc)[None, :, None], idx.shape)
        ci = np.broadcast_to(np.arange(Cc)[None, None, :], idx.shape)
        win = (sat[bi, ci, h2, w2_] - sat[bi, ci, h1, w2_]
               - sat[bi, ci, h2, w1_] + sat[bi, ci, h1, w1_])
        k = 2 * RADIUS + 1
        cm = win / (k * k)
        return cm.mean(0).mean()

    k0, k1, k2 = jax.random.split(jax.random.key(42), 3)
    a1 = act_value(cosine_field(y1, w1, s1), k0)
    a2 = act_value(cosine_field(y2, w2, s2), k1)
    a3 = act_value(cosine_field(y3, w3, s3), k2)
    return np.float32(a1 + a2 + a3)


def run_device(inputs, trace=False, trace_cores=None, stitch=False):
    x = np.ascontiguousarray(np.asarray(inputs["x"], np.float32).reshape(B, C1, HW))
    w1e, w2p, w2s, w3e, g1b1, g2b2, g3b3 = _prep_weights(
        *[np.asarray(inputs[k], np.float32) for k in
          ["w1", "w2", "w3", "g1", "b1", "g2", "b2", "g3", "b3"]])

    nc = build_nc()
    in_maps = []
    for c in range(NCORES):
        in_maps.append({
            "xin": x[BLOC * c:BLOC * (c + 1)],
            "w1e": w1e, "w2p": w2p, "w2s": w2s, "w3e": w3e,
            "g1b1": g1b1, "g2b2": g2b2, "g3b3": g3b3,
        })
    res = run_bass_kernel_spmd(nc, in_maps, list(range(NCORES)), trace=trace,
                               trace_cores=trace_cores, stitch_traces=stitch)
    cat = lambda key: np.concatenate([res.results[c][key] for c in range(NCORES)], 0)
    fields = {k: cat(k) for k in ["y1o", "y2o", "y3o", "outo"]}
    return fields, res


def kernel(**inputs):
    x = np.asarray(inputs["x"], np.float32)
    w1 = np.asarray(inputs["w1"], np.float32)
    w2 = np.asarray(inputs["w2"], np.float32)
    w3 = np.asarray(inputs["w3"], np.float32)

    fields, _ = run_device(inputs)
    out = fields["outo"].reshape(B, C3, H, W)
    acc = _host_acc(x, w1, w2, w3,
                    np.asarray(inputs["g1"], np.float32),
                    np.asarray(inputs["b1"], np.float32),
                    np.asarray(inputs["g2"], np.float32),
                    np.asarray(inputs["b2"], np.float32),
                    fields["y1o"], fields["y2o"], fields["y3o"])
    return out, acc
